# revision 19
# baseline (speedup 1.0000x reference)
"""Distributed Bass kernel for nn_Attention (LN -> QKV -> MHA -> out-proj).

Sharding (8 cores, SPMD-uniform graph):
  - core i computes heads {2i, 2i+1} for BOTH batches (tensor-parallel on heads)
  - per-head AllToAll redistributes head-channels -> token slices (2 collectives,
    first overlaps second head's attention); core i finishes the out-projection
    for global tokens [512*i, 512*(i+1)) (batch i//4, rows 512*(i%4)...)

Device pipeline per core:
  A. x (bf16) [tok, ch] -> bn_stats -> per-partition (x-mu)*rstd on GpSimd ->
     xn staged to DRAM -> XBAR transpose DMAs -> xn^T [ch, tok]
  B. QKV: qT/kT = W^T-slice @ xn^T (+bias, DVE evac); v in [tok, ch] (+bias via
     K=1 ones matmul); qT/kT duplicated into both 64-partition halves (enables
     k-tile pairing via tile_position row groups)
  C. per (head, batch): S^T = kT.T@qT row-group-paired k-tiles; exp on ScalarE
     (scale=Dh^-0.5 folded, no max-subtraction); O^T = [v|1].T @ P^T -> row 64
     = softmax denominator; psS bufs=3 / psO bufs=2 keep ACT and PE pipelined
  D. per-head AllToAll (bf16): shard j = [64 O^T rows + denom, 512 toks of j]
  E. normalize with gathered denominators (compact reciprocal + bcast DMA)
  F. y = xa @ W_out + b_out for my 512 tokens -> out
"""

import sys

sys.path.insert(0, "/opt/trn_rl_repo")

import numpy as np
import ml_dtypes

DIM = 1024
HEADS = 16
B = 2
N = 2048
Dh = 64
NCORES = 8
T = B * N  # 4096 global tokens
HPC = 2  # heads per core
CHC = HPC * Dh  # 128 channels per core
SCALE = Dh**-0.5
BF16 = ml_dtypes.bfloat16

_cache = {}


def _build():
    import concourse.bass as bass
    import concourse.tile as tile
    from concourse import bacc, mybir

    fp32 = mybir.dt.float32
    bf16 = mybir.dt.bfloat16
    AF = mybir.ActivationFunctionType
    OP = mybir.AluOpType

    nc = bacc.Bacc("TRN2", target_bir_lowering=False, debug=False, num_devices=NCORES)

    x_ext = nc.dram_tensor("x", [T, DIM], bf16, kind="ExternalInput")
    wq_ext = nc.dram_tensor("wq", [DIM, CHC], bf16, kind="ExternalInput")
    wk_ext = nc.dram_tensor("wk", [DIM, CHC], bf16, kind="ExternalInput")
    wv_ext = nc.dram_tensor("wv", [DIM, CHC], bf16, kind="ExternalInput")
    bqk_ext = nc.dram_tensor("bqk", [128, 2], fp32, kind="ExternalInput")
    bv_ext = nc.dram_tensor("bv", [1, CHC], bf16, kind="ExternalInput")
    wo_ext = nc.dram_tensor("wo", [DIM, DIM], bf16, kind="ExternalInput")
    bo_ext = nc.dram_tensor("bo", [1, DIM], fp32, kind="ExternalInput")
    out_ext = nc.dram_tensor("out", [512, DIM], fp32, kind="ExternalOutput")

    NT = T // 128  # 32 token tiles
    NC = DIM // 128  # 8 channel chunks
    NKT = N // 128  # 16 k-tiles per batch

    with tile.TileContext(nc) as tc:
        with (
            tc.tile_pool(name="persist", bufs=1) as persist,
            tc.tile_pool(name="dram", bufs=1, space="DRAM") as dram,
        ):
            eps_ap = persist.tile([128, 1], fp32, tag="eps")
            nc.vector.memset(eps_ap, 1e-5)
            ones_col = persist.tile([1, 128], bf16, tag="ones_col")
            nc.vector.memset(ones_col, 1.0)

            # weights: SWDGE (gpsimd) queues so they don't contend with x loads
            wq_sb = persist.tile([128, NC, CHC], bf16, tag="wq")
            wk_sb = persist.tile([128, NC, CHC], bf16, tag="wk")
            wv_sb = persist.tile([128, NC, CHC], bf16, tag="wv")
            wo_sb = persist.tile([128, NC, DIM], bf16, tag="wo")
            bqk_sb = persist.tile([128, 2], fp32, tag="bqk")
            bv_sb = persist.tile([1, CHC], bf16, tag="bv")
            bo_sb = persist.tile([128, DIM], fp32, tag="bo")
            nc.gpsimd.dma_start(out=wq_sb, in_=wq_ext.ap().rearrange("(c p) m -> p c m", p=128))
            nc.gpsimd.dma_start(out=wk_sb, in_=wk_ext.ap().rearrange("(c p) m -> p c m", p=128))
            nc.gpsimd.dma_start(out=wv_sb, in_=wv_ext.ap().rearrange("(c p) m -> p c m", p=128))
            nc.gpsimd.dma_start(out=wo_sb, in_=wo_ext.ap().rearrange("(c p) m -> p c m", p=128))
            nc.gpsimd.dma_start(out=bqk_sb, in_=bqk_ext.ap())
            nc.gpsimd.dma_start(out=bv_sb, in_=bv_ext.ap())
            nc.gpsimd.dma_start(out=bo_sb, in_=bo_ext.ap().to_broadcast((128, DIM)))

            # persistent activations
            qT2 = [persist.tile([128, T], bf16, tag=f"qT2_{h}", name=f"qT2_{h}") for h in range(HPC)]
            kT2 = [persist.tile([128, T], bf16, tag=f"kT2_{h}", name=f"kT2_{h}") for h in range(HPC)]
            v_ext_t = persist.tile([128, NT, HPC, 72], bf16, tag="v_ext")
            nc.vector.memset(v_ext_t[:, :, :, 64:65], 1.0)
            OTn = [persist.tile([65, N], bf16, tag=f"OTn_{u}", name=f"OTn_{u}") for u in range(B * HPC)]

            xnT = persist.tile([128, NC, T], bf16, tag="xnT")

            # A2A bounce buffers, one pair per head slot
            in_b = [dram.tile([NCORES * 65, 512], bf16, name=f"in_b{h}") for h in range(HPC)]
            out_b = [dram.tile([NCORES * 65, 512], bf16, name=f"out_b{h}") for h in range(HPC)]

            # ---------------- Phase A: load + LN + transpose ----------------
            with (
                tc.tile_pool(name="xpool", bufs=8) as xpool,
                tc.tile_pool(name="psA", bufs=4, space="PSUM") as psA,
            ):
                xstage = [dram.tile([N // 2, DIM], bf16, name=f"xstage{r}") for r in range(4)]
                for t in range(NT):
                    x_t = xpool.tile([128, DIM], bf16, tag="x_t")
                    nc.gpsimd.dma_start(out=x_t, in_=x_ext.ap()[t * 128 : (t + 1) * 128, :])
                    st = xpool.tile([128, 2, 6], fp32, tag="bn_st")
                    nc.vector.bn_stats(out=st[:, 0, :], in_=x_t[:, 0:512])
                    nc.vector.bn_stats(out=st[:, 1, :], in_=x_t[:, 512:1024])
                    mv = xpool.tile([128, 2], fp32, tag="bn_mv")
                    nc.vector.bn_aggr(out=mv, in_=st)
                    rstd_t = xpool.tile([128, 1], fp32, tag="rstd_t")
                    nc.scalar.activation(out=rstd_t, in_=mv[:, 1:2], func=AF.Sqrt, bias=eps_ap, scale=1.0)
                    nc.vector.reciprocal(out=rstd_t, in_=rstd_t)
                    xn_t = xpool.tile([128, DIM], bf16, tag="xn_t")
                    nc.vector.tensor_scalar(
                        out=xn_t, in0=x_t, scalar1=mv[:, 0:1], scalar2=rstd_t,
                        op0=OP.subtract, op1=OP.mult,
                    )
                    r, tt = t // 8, t % 8
                    nc.gpsimd.dma_start(out=xstage[r][tt * 128 : (tt + 1) * 128, :], in_=xn_t)
                    if tt == 7:  # row-group staged: transpose it (sync queue is transpose-only)
                        for c in range(NC):
                            nc.sync.dma_start_transpose(
                                xnT[:, c, r * 1024 : (r + 1) * 1024],
                                xstage[r][:, c * 128 : (c + 1) * 128],
                            )

                # ---------------- Phase B: QKV (per batch, so attention starts early) ----------------
                qT_t = persist.tile([128, T], bf16, tag="qT_t")
                kT_t = persist.tile([128, T], bf16, tag="kT_t")
                for bt in range(B):
                    for (w_sb, dst, bcol) in ((wq_sb, qT_t, 0), (wk_sb, kT_t, 1)):
                        for tc4 in range(bt * 4, bt * 4 + 4):
                            ps = psA.tile([128, 512], fp32, tag="ps_qkv")
                            for c in range(NC):
                                nc.tensor.matmul(
                                    ps, w_sb[:, c, :], xnT[:, c, tc4 * 512 : (tc4 + 1) * 512],
                                    start=(c == 0), stop=(c == NC - 1),
                                )
                            nc.vector.tensor_scalar(
                                out=dst[:, tc4 * 512 : (tc4 + 1) * 512], in0=ps,
                                scalar1=bqk_sb[:, bcol : bcol + 1], scalar2=None,
                                op0=OP.add,
                            )
                    for t in range(bt * 16, bt * 16 + 16):
                        ps = psA.tile([128, CHC], fp32, tag="ps_qkv")
                        nc.tensor.matmul(ps, ones_col, bv_sb, start=True, stop=False)
                        for c in range(NC):
                            nc.tensor.matmul(
                                ps, xnT[:, c, t * 128 : (t + 1) * 128], wv_sb[:, c, :],
                                start=False, stop=(c == NC - 1),
                            )
                        nc.vector.tensor_copy(
                            out=v_ext_t[:, t, :, 0:64],
                            in_=ps.rearrange("p (h d) -> p h d", h=HPC),
                        )
                    for h in range(HPC):
                        s0, s1 = bt * N, (bt + 1) * N
                        src_q = qT_t[h * 64 : (h + 1) * 64, s0:s1]
                        src_k = kT_t[h * 64 : (h + 1) * 64, s0:s1]
                        nc.sync.dma_start(out=qT2[h][0:64, s0:s1], in_=src_q)
                        nc.sync.dma_start(out=qT2[h][64:128, s0:s1], in_=src_q)
                        nc.sync.dma_start(out=kT2[h][0:64, s0:s1], in_=src_k)
                        nc.sync.dma_start(out=kT2[h][64:128, s0:s1], in_=src_k)

            # ---------------- Phase C: attention (+ per-head A2A) ----------------
            with (
                tc.tile_pool(name="pt", bufs=3) as ptpool,
                tc.tile_pool(name="psS", bufs=3, space="PSUM") as psS,
                tc.tile_pool(name="psO", bufs=2, space="PSUM") as psO,
            ):
                for h in range(HPC):
                    for bt in range(B):
                        u = bt * HPC + h
                        tok0 = bt * N
                        kt0 = bt * NKT
                        for qc in range(4):
                            q0 = tok0 + qc * 512
                            ps_o = psO.tile([128, 512], fp32, tag="ps_o")
                            for kp in range(NKT // 2):
                                ps_s = psS.tile([128, 2, 512], fp32, tag="ps_s")
                                for d in range(2):
                                    kt = 2 * kp + d
                                    lo = d * 64
                                    nc.tensor.matmul(
                                        ps_s[:, d, :],
                                        kT2[h][lo : lo + 64, tok0 + kt * 128 : tok0 + (kt + 1) * 128],
                                        qT2[h][lo : lo + 64, q0 : q0 + 512],
                                        start=True, stop=True,
                                        tile_position=(lo, 0),
                                    )
                                pt_t = ptpool.tile([128, 2, 512], bf16, tag="pt")
                                nc.scalar.activation(out=pt_t, in_=ps_s, func=AF.Exp, scale=SCALE)
                                for d in range(2):
                                    kt = 2 * kp + d
                                    nc.tensor.matmul(
                                        ps_o[0:65, :],
                                        v_ext_t[:, kt0 + kt, h, 0:65],
                                        pt_t[:, d, :],
                                        start=(kp == 0 and d == 0),
                                        stop=(kp == NKT // 2 - 1 and d == 1),
                                    )
                            nc.vector.tensor_copy(
                                out=OTn[u][:, qc * 512 : (qc + 1) * 512], in_=ps_o[0:65, :]
                            )
                            j = bt * 4 + qc  # A2A shard fed by this (unit, qc)
                            nc.sync.dma_start(
                                out=in_b[h][j * 65 : j * 65 + 65, :],
                                in_=OTn[u][:, qc * 512 : (qc + 1) * 512],
                            )
                    nc.gpsimd.collective_compute(
                        "AllToAll",
                        mybir.AluOpType.bypass,
                        replica_groups=[list(range(NCORES))],
                        ins=[in_b[h].opt()],
                        outs=[out_b[h].opt()],
                    )

            # ---------------- Phase E/F: normalize + out-proj ----------------
            with (
                tc.tile_pool(name="fin", bufs=1) as fin,
                tc.tile_pool(name="psY", bufs=1, space="PSUM") as psY,
            ):
                ps_y_all = [None] * 8
                rcp_dram = [dram.tile([NC, 512], bf16, name=f"rcp_dram{h}") for h in range(HPC)]
                xa_raw = fin.tile([128, NC, 512], bf16, tag="xa_raw")
                dnm_b = fin.tile([128, NC, 512], bf16, tag="dnm_b")
                xa = fin.tile([128, NC, 512], bf16, tag="xa")
                for h in range(HPC):
                    # O^T rows: one batched DMA [64, NC, 512]
                    nc.sync.dma_start(
                        out=xa_raw[h * 64 : (h + 1) * 64, :, :],
                        in_=out_b[h].rearrange("(c r) t -> r c t", r=65)[0:64, :, :],
                    )
                    # compact denominators -> [64, 64] (chunk cc at partitions 8cc..)
                    dn_c = fin.tile([64, 64], bf16, tag=f"dn_c{h}", name=f"dn_c{h}")
                    for cc in range(NC):
                        nc.sync.dma_start(
                            out=dn_c[cc * 8 : (cc + 1) * 8, :],
                            in_=out_b[h][cc * 65 + 64 : cc * 65 + 65, :].rearrange(
                                "o (a b) -> (o a) b", a=8
                            ),
                        )
                    rcp_f = fin.tile([64, 64], fp32, tag=f"rcp_f{h}", name=f"rcp_f{h}")
                    nc.vector.reciprocal(out=rcp_f, in_=dn_c)
                    rcp_bf = fin.tile([64, 64], bf16, tag=f"rcp_bf{h}", name=f"rcp_bf{h}")
                    nc.vector.tensor_copy(out=rcp_bf, in_=rcp_f)
                    nc.sync.dma_start(
                        out=rcp_dram[h].rearrange("c (a b) -> (c a) b", a=8), in_=rcp_bf
                    )
                    # broadcast each chunk's row across 64 partitions: one DMA
                    nc.sync.dma_start(
                        out=dnm_b[h * 64 : (h + 1) * 64, :, :],
                        in_=rcp_dram[h][None, :, :].to_broadcast((64, NC, 512)),
                    )
                    nc.vector.tensor_tensor(
                        xa[h * 64 : (h + 1) * 64, :, :],
                        xa_raw[h * 64 : (h + 1) * 64, :, :],
                        dnm_b[h * 64 : (h + 1) * 64, :, :],
                        OP.mult,
                    )
                    # out-proj pass for this head's 64 channels per chunk
                    # (row group h*64; h=0 pass runs during the second A2A)
                    lo = h * 64
                    for mt in range(4):
                        for nh in range(2):
                            ps = psY.tile(
                                [128, 512], fp32, tag=f"ps_y{mt}{nh}", name=f"ps_y{mt}{nh}{h}"
                            )
                            ps_y_all[mt * 2 + nh] = ps
                            for c in range(NC):
                                nc.tensor.matmul(
                                    ps,
                                    xa[lo : lo + 64, c, mt * 128 : (mt + 1) * 128],
                                    wo_sb[lo : lo + 64, c, nh * 512 : (nh + 1) * 512],
                                    start=(c == 0 and h == 0),
                                    stop=(c == NC - 1 and h == HPC - 1),
                                    tile_position=(lo, 0),
                                )

                for mt in range(4):
                    y = fin.tile([128, DIM], fp32, tag="y")
                    for nh in range(2):
                        nc.vector.tensor_tensor(
                            y[:, nh * 512 : (nh + 1) * 512], ps_y_all[mt * 2 + nh],
                            bo_sb[:, nh * 512 : (nh + 1) * 512], OP.add,
                        )
                    nc.sync.dma_start(
                        out=out_ext.ap()[mt * 128 : (mt + 1) * 128, :], in_=y
                    )

    nc.compile()
    return nc


def _prep_inputs(x, ln_gamma, ln_beta, W_qkv, W_out, b_out):
    """Host-side: fold gamma/beta into W_qkv, slice per core, cast to bf16."""
    Wf = ln_gamma[:, None].astype(np.float64) * W_qkv.astype(np.float64)
    bf = ln_beta.astype(np.float64) @ W_qkv.astype(np.float64)  # [3*DIM]
    x_all = x.reshape(T, DIM).astype(BF16)
    wo = W_out.astype(BF16)
    bo = b_out.astype(np.float32).reshape(1, DIM)
    in_maps = []
    for i in range(NCORES):
        c0 = i * CHC  # channel block of this core's 2 heads
        wq = Wf[:, 0 * DIM + c0 : 0 * DIM + c0 + CHC]
        wk = Wf[:, 1 * DIM + c0 : 1 * DIM + c0 + CHC]
        wv = Wf[:, 2 * DIM + c0 : 2 * DIM + c0 + CHC]
        bq = bf[0 * DIM + c0 : 0 * DIM + c0 + CHC]
        bk = bf[1 * DIM + c0 : 1 * DIM + c0 + CHC]
        bv = bf[2 * DIM + c0 : 2 * DIM + c0 + CHC]
        bqk = np.stack([bq, bk], axis=1).astype(np.float32)  # [128, 2]
        in_maps.append(
            {
                "x": x_all,
                "wq": np.ascontiguousarray(wq.astype(BF16)),
                "wk": np.ascontiguousarray(wk.astype(BF16)),
                "wv": np.ascontiguousarray(wv.astype(BF16)),
                "bqk": np.ascontiguousarray(bqk),
                "bv": np.ascontiguousarray(bv.astype(BF16).reshape(1, CHC)),
                "wo": wo,
                "bo": bo,
            }
        )
    return in_maps


def kernel(x, ln_gamma, ln_beta, W_qkv, W_out, b_out, _want_time=False):
    x = np.asarray(x, dtype=np.float32)
    ln_gamma = np.asarray(ln_gamma, dtype=np.float32)
    ln_beta = np.asarray(ln_beta, dtype=np.float32)
    W_qkv = np.asarray(W_qkv, dtype=np.float32)
    W_out = np.asarray(W_out, dtype=np.float32)
    b_out = np.asarray(b_out, dtype=np.float32)

    if "nc" not in _cache:
        _cache["nc"] = _build()
    nc = _cache["nc"]

    from concourse.bass_utils import run_bass_kernel_spmd

    in_maps = _prep_inputs(x, ln_gamma, ln_beta, W_qkv, W_out, b_out)
    res = run_bass_kernel_spmd(
        nc, in_maps, core_ids=list(range(NCORES)), trace=_want_time
    )
    out = np.empty((B, N, DIM), dtype=np.float32)
    for i in range(NCORES):
        b, g = i // 4, i % 4
        out[b, g * 512 : (g + 1) * 512, :] = res.results[i]["out"]
    if _want_time:
        return out, res.exec_time_ns
    return out


# revision 20
# speedup vs baseline: 1.2535x; 1.2535x over previous
"""Distributed Bass kernel for nn_Attention (LN -> QKV -> MHA -> out-proj).

Sharding (8 cores, SPMD-uniform graph):
  - core i computes heads {2i, 2i+1} for BOTH batches (tensor-parallel on heads)
  - per-head AllToAll redistributes head-channels -> token slices (2 collectives,
    first overlaps second head's attention); core i finishes the out-projection
    for global tokens [512*i, 512*(i+1)) (batch i//4, rows 512*(i%4)...)

Device pipeline per core:
  A. x (bf16) [tok, ch] -> bn_stats -> per-partition (x-mu)*rstd on GpSimd ->
     xn staged to DRAM -> XBAR transpose DMAs -> xn^T [ch, tok]
  B. QKV: qT/kT = W^T-slice @ xn^T (+bias, DVE evac); v in [tok, ch] (+bias via
     K=1 ones matmul); qT/kT duplicated into both 64-partition halves (enables
     k-tile pairing via tile_position row groups)
  C. per (head, batch): S^T = kT.T@qT row-group-paired k-tiles; exp on ScalarE
     (scale=Dh^-0.5 folded, no max-subtraction); O^T = [v|1].T @ P^T -> row 64
     = softmax denominator; psS bufs=3 / psO bufs=2 keep ACT and PE pipelined
  D. per-head AllToAll (bf16): shard j = [64 O^T rows + denom, 512 toks of j]
  E. normalize with gathered denominators (compact reciprocal + bcast DMA)
  F. y = xa @ W_out + b_out for my 512 tokens -> out
"""

import sys

sys.path.insert(0, "/opt/trn_rl_repo")

import numpy as np
import ml_dtypes

DIM = 1024
HEADS = 16
B = 2
N = 2048
Dh = 64
NCORES = 8
T = B * N  # 4096 global tokens
HPC = 2  # heads per core
CHC = HPC * Dh  # 128 channels per core
SCALE = Dh**-0.5
BF16 = ml_dtypes.bfloat16

_cache = {}


def _build():
    import concourse.bass as bass
    import concourse.tile as tile
    from concourse import bacc, mybir

    fp32 = mybir.dt.float32
    bf16 = mybir.dt.bfloat16
    AF = mybir.ActivationFunctionType
    OP = mybir.AluOpType

    nc = bacc.Bacc("TRN2", target_bir_lowering=False, debug=False, num_devices=NCORES)

    x_ext = nc.dram_tensor("x", [T, DIM], bf16, kind="ExternalInput")
    wq_ext = nc.dram_tensor("wq", [DIM, CHC], bf16, kind="ExternalInput")
    wk_ext = nc.dram_tensor("wk", [DIM, CHC], bf16, kind="ExternalInput")
    wv_ext = nc.dram_tensor("wv", [DIM, CHC], bf16, kind="ExternalInput")
    bqk_ext = nc.dram_tensor("bqk", [128, 2], fp32, kind="ExternalInput")
    bv_ext = nc.dram_tensor("bv", [1, CHC], bf16, kind="ExternalInput")
    wo_ext = nc.dram_tensor("wo", [DIM, DIM], bf16, kind="ExternalInput")
    bo_ext = nc.dram_tensor("bo", [1, DIM], fp32, kind="ExternalInput")
    out_ext = nc.dram_tensor("out", [512, DIM], fp32, kind="ExternalOutput")

    NT = T // 128  # 32 token tiles
    NC = DIM // 128  # 8 channel chunks
    NKT = N // 128  # 16 k-tiles per batch

    with tile.TileContext(nc) as tc:
        with (
            tc.tile_pool(name="persist", bufs=1) as persist,
            tc.tile_pool(name="dram", bufs=1, space="DRAM") as dram,
        ):
            eps_ap = persist.tile([128, 1], fp32, tag="eps")
            nc.vector.memset(eps_ap, 1e-5)
            ones_col = persist.tile([1, 128], bf16, tag="ones_col")
            nc.vector.memset(ones_col, 1.0)

            # weights: SWDGE (gpsimd) queues so they don't contend with x loads
            wq_sb = persist.tile([128, NC, CHC], bf16, tag="wq")
            wk_sb = persist.tile([128, NC, CHC], bf16, tag="wk")
            wv_sb = persist.tile([128, NC, CHC], bf16, tag="wv")
            wo_sb = persist.tile([128, NC, DIM], bf16, tag="wo")
            bqk_sb = persist.tile([128, 2], fp32, tag="bqk")
            bv_sb = persist.tile([1, CHC], bf16, tag="bv")
            bo_sb = persist.tile([128, DIM], fp32, tag="bo")
            nc.gpsimd.dma_start(out=wq_sb, in_=wq_ext.ap().rearrange("(c p) m -> p c m", p=128))
            nc.gpsimd.dma_start(out=wk_sb, in_=wk_ext.ap().rearrange("(c p) m -> p c m", p=128))
            nc.gpsimd.dma_start(out=wv_sb, in_=wv_ext.ap().rearrange("(c p) m -> p c m", p=128))
            nc.gpsimd.dma_start(out=wo_sb, in_=wo_ext.ap().rearrange("(c p) m -> p c m", p=128))
            nc.gpsimd.dma_start(out=bqk_sb, in_=bqk_ext.ap())
            nc.gpsimd.dma_start(out=bv_sb, in_=bv_ext.ap())
            nc.gpsimd.dma_start(out=bo_sb, in_=bo_ext.ap().to_broadcast((128, DIM)))

            # persistent activations
            qT2 = [persist.tile([128, T], bf16, tag=f"qT2_{h}", name=f"qT2_{h}") for h in range(HPC)]
            kT2 = [persist.tile([128, T], bf16, tag=f"kT2_{h}", name=f"kT2_{h}") for h in range(HPC)]
            v_ext_t = persist.tile([128, NT, HPC, 72], bf16, tag="v_ext")
            nc.vector.memset(v_ext_t[:, :, :, 64:65], 1.0)
            OTn = [persist.tile([65, N], bf16, tag=f"OTn_{u}", name=f"OTn_{u}") for u in range(B * HPC)]

            xnT = persist.tile([128, NC, T], bf16, tag="xnT")

            # A2A bounce buffers, one pair per head slot
            in_b = [dram.tile([NCORES * 65, 512], bf16, name=f"in_b{h}") for h in range(HPC)]
            out_b = [dram.tile([NCORES * 65, 512], bf16, name=f"out_b{h}") for h in range(HPC)]

            # ---------------- Phase A: load + LN + transpose ----------------
            with (
                tc.tile_pool(name="xpool", bufs=8) as xpool,
                tc.tile_pool(name="psA", bufs=4, space="PSUM") as psA,
            ):
                xstage = [dram.tile([N // 2, DIM], bf16, name=f"xstage{r}") for r in range(4)]
                for t in range(NT):
                    x_t = xpool.tile([128, DIM], bf16, tag="x_t")
                    nc.sync.dma_start(out=x_t, in_=x_ext.ap()[t * 128 : (t + 1) * 128, :])
                    st = xpool.tile([128, 2, 6], fp32, tag="bn_st")
                    nc.vector.bn_stats(out=st[:, 0, :], in_=x_t[:, 0:512])
                    nc.vector.bn_stats(out=st[:, 1, :], in_=x_t[:, 512:1024])
                    mv = xpool.tile([128, 2], fp32, tag="bn_mv")
                    nc.vector.bn_aggr(out=mv, in_=st)
                    rstd_t = xpool.tile([128, 1], fp32, tag="rstd_t")
                    nc.scalar.activation(out=rstd_t, in_=mv[:, 1:2], func=AF.Sqrt, bias=eps_ap, scale=1.0)
                    nc.vector.reciprocal(out=rstd_t, in_=rstd_t)
                    xn_t = xpool.tile([128, DIM], bf16, tag="xn_t")
                    nc.vector.tensor_scalar(
                        out=xn_t, in0=x_t, scalar1=mv[:, 0:1], scalar2=rstd_t,
                        op0=OP.subtract, op1=OP.mult,
                    )
                    r, tt = t // 8, t % 8
                    nc.sync.dma_start(out=xstage[r][tt * 128 : (tt + 1) * 128, :], in_=xn_t)
                    if tt == 7:  # row-group staged: transpose it (sync queue is transpose-only)
                        for c in range(NC):
                            nc.sync.dma_start_transpose(
                                xnT[:, c, r * 1024 : (r + 1) * 1024],
                                xstage[r][:, c * 128 : (c + 1) * 128],
                            )

                # ---------------- Phase B: QKV (per batch, so attention starts early) ----------------
                qT_t = persist.tile([128, T], bf16, tag="qT_t")
                kT_t = persist.tile([128, T], bf16, tag="kT_t")
                for bt in range(B):
                    for (w_sb, dst, bcol) in ((wq_sb, qT_t, 0), (wk_sb, kT_t, 1)):
                        for tc4 in range(bt * 4, bt * 4 + 4):
                            ps = psA.tile([128, 512], fp32, tag="ps_qkv")
                            for c in range(NC):
                                nc.tensor.matmul(
                                    ps, w_sb[:, c, :], xnT[:, c, tc4 * 512 : (tc4 + 1) * 512],
                                    start=(c == 0), stop=(c == NC - 1),
                                )
                            nc.vector.tensor_scalar(
                                out=dst[:, tc4 * 512 : (tc4 + 1) * 512], in0=ps,
                                scalar1=bqk_sb[:, bcol : bcol + 1], scalar2=None,
                                op0=OP.add,
                            )
                    for t in range(bt * 16, bt * 16 + 16):
                        ps = psA.tile([128, CHC], fp32, tag="ps_qkv")
                        nc.tensor.matmul(ps, ones_col, bv_sb, start=True, stop=False)
                        for c in range(NC):
                            nc.tensor.matmul(
                                ps, xnT[:, c, t * 128 : (t + 1) * 128], wv_sb[:, c, :],
                                start=False, stop=(c == NC - 1),
                            )
                        nc.vector.tensor_copy(
                            out=v_ext_t[:, t, :, 0:64],
                            in_=ps.rearrange("p (h d) -> p h d", h=HPC),
                        )
                    for h in range(HPC):
                        s0, s1 = bt * N, (bt + 1) * N
                        src_q = qT_t[h * 64 : (h + 1) * 64, s0:s1]
                        src_k = kT_t[h * 64 : (h + 1) * 64, s0:s1]
                        nc.sync.dma_start(out=qT2[h][0:64, s0:s1], in_=src_q)
                        nc.sync.dma_start(out=qT2[h][64:128, s0:s1], in_=src_q)
                        nc.sync.dma_start(out=kT2[h][0:64, s0:s1], in_=src_k)
                        nc.sync.dma_start(out=kT2[h][64:128, s0:s1], in_=src_k)

            # ---------------- Phase C: attention (+ per-head A2A) ----------------
            with (
                tc.tile_pool(name="pt", bufs=3) as ptpool,
                tc.tile_pool(name="psS", bufs=3, space="PSUM") as psS,
                tc.tile_pool(name="psO", bufs=2, space="PSUM") as psO,
            ):
                for h in range(HPC):
                    for bt in range(B):
                        u = bt * HPC + h
                        tok0 = bt * N
                        kt0 = bt * NKT
                        for qc in range(4):
                            q0 = tok0 + qc * 512
                            ps_o = psO.tile([128, 512], fp32, tag="ps_o")
                            for kp in range(NKT // 2):
                                ps_s = psS.tile([128, 2, 512], fp32, tag="ps_s")
                                for d in range(2):
                                    kt = 2 * kp + d
                                    lo = d * 64
                                    nc.tensor.matmul(
                                        ps_s[:, d, :],
                                        kT2[h][lo : lo + 64, tok0 + kt * 128 : tok0 + (kt + 1) * 128],
                                        qT2[h][lo : lo + 64, q0 : q0 + 512],
                                        start=True, stop=True,
                                        tile_position=(lo, 0),
                                    )
                                pt_t = ptpool.tile([128, 2, 512], bf16, tag="pt")
                                nc.scalar.activation(out=pt_t, in_=ps_s, func=AF.Exp, scale=SCALE)
                                for d in range(2):
                                    kt = 2 * kp + d
                                    nc.tensor.matmul(
                                        ps_o[0:65, :],
                                        v_ext_t[:, kt0 + kt, h, 0:65],
                                        pt_t[:, d, :],
                                        start=(kp == 0 and d == 0),
                                        stop=(kp == NKT // 2 - 1 and d == 1),
                                    )
                            nc.vector.tensor_copy(
                                out=OTn[u][:, qc * 512 : (qc + 1) * 512], in_=ps_o[0:65, :]
                            )
                            j = bt * 4 + qc  # A2A shard fed by this (unit, qc)
                            nc.sync.dma_start(
                                out=in_b[h][j * 65 : j * 65 + 65, :],
                                in_=OTn[u][:, qc * 512 : (qc + 1) * 512],
                            )
                    nc.gpsimd.collective_compute(
                        "AllToAll",
                        mybir.AluOpType.bypass,
                        replica_groups=[list(range(NCORES))],
                        ins=[in_b[h].opt()],
                        outs=[out_b[h].opt()],
                    )

            # ---------------- Phase E/F: normalize + out-proj ----------------
            with (
                tc.tile_pool(name="fin", bufs=1) as fin,
                tc.tile_pool(name="psY", bufs=1, space="PSUM") as psY,
            ):
                ps_y_all = [None] * 8
                rcp_dram = [dram.tile([NC, 512], bf16, name=f"rcp_dram{h}") for h in range(HPC)]
                xa_raw = fin.tile([128, NC, 512], bf16, tag="xa_raw")
                dnm_b = fin.tile([128, NC, 512], bf16, tag="dnm_b")
                xa = fin.tile([128, NC, 512], bf16, tag="xa")
                for h in range(HPC):
                    # O^T rows: one batched DMA [64, NC, 512]
                    nc.sync.dma_start(
                        out=xa_raw[h * 64 : (h + 1) * 64, :, :],
                        in_=out_b[h].rearrange("(c r) t -> r c t", r=65)[0:64, :, :],
                    )
                    # compact denominators -> [64, 64] (chunk cc at partitions 8cc..)
                    dn_c = fin.tile([64, 64], bf16, tag=f"dn_c{h}", name=f"dn_c{h}")
                    for cc in range(NC):
                        nc.sync.dma_start(
                            out=dn_c[cc * 8 : (cc + 1) * 8, :],
                            in_=out_b[h][cc * 65 + 64 : cc * 65 + 65, :].rearrange(
                                "o (a b) -> (o a) b", a=8
                            ),
                        )
                    rcp_f = fin.tile([64, 64], fp32, tag=f"rcp_f{h}", name=f"rcp_f{h}")
                    nc.vector.reciprocal(out=rcp_f, in_=dn_c)
                    rcp_bf = fin.tile([64, 64], bf16, tag=f"rcp_bf{h}", name=f"rcp_bf{h}")
                    nc.vector.tensor_copy(out=rcp_bf, in_=rcp_f)
                    nc.sync.dma_start(
                        out=rcp_dram[h].rearrange("c (a b) -> (c a) b", a=8), in_=rcp_bf
                    )
                    # broadcast each chunk's row across 64 partitions: one DMA
                    nc.sync.dma_start(
                        out=dnm_b[h * 64 : (h + 1) * 64, :, :],
                        in_=rcp_dram[h][None, :, :].to_broadcast((64, NC, 512)),
                    )
                    nc.vector.tensor_tensor(
                        xa[h * 64 : (h + 1) * 64, :, :],
                        xa_raw[h * 64 : (h + 1) * 64, :, :],
                        dnm_b[h * 64 : (h + 1) * 64, :, :],
                        OP.mult,
                    )
                    # out-proj pass for this head's 64 channels per chunk
                    # (row group h*64; h=0 pass runs during the second A2A)
                    lo = h * 64
                    for mt in range(4):
                        for nh in range(2):
                            ps = psY.tile(
                                [128, 512], fp32, tag=f"ps_y{mt}{nh}", name=f"ps_y{mt}{nh}{h}"
                            )
                            ps_y_all[mt * 2 + nh] = ps
                            for c in range(NC):
                                nc.tensor.matmul(
                                    ps,
                                    xa[lo : lo + 64, c, mt * 128 : (mt + 1) * 128],
                                    wo_sb[lo : lo + 64, c, nh * 512 : (nh + 1) * 512],
                                    start=(c == 0 and h == 0),
                                    stop=(c == NC - 1 and h == HPC - 1),
                                    tile_position=(lo, 0),
                                )

                for mt in range(4):
                    y = fin.tile([128, DIM], fp32, tag="y")
                    for nh in range(2):
                        nc.vector.tensor_tensor(
                            y[:, nh * 512 : (nh + 1) * 512], ps_y_all[mt * 2 + nh],
                            bo_sb[:, nh * 512 : (nh + 1) * 512], OP.add,
                        )
                    nc.sync.dma_start(
                        out=out_ext.ap()[mt * 128 : (mt + 1) * 128, :], in_=y
                    )

    nc.compile()
    return nc


def _prep_inputs(x, ln_gamma, ln_beta, W_qkv, W_out, b_out):
    """Host-side: fold gamma/beta into W_qkv, slice per core, cast to bf16."""
    Wf = ln_gamma[:, None].astype(np.float64) * W_qkv.astype(np.float64)
    bf = ln_beta.astype(np.float64) @ W_qkv.astype(np.float64)  # [3*DIM]
    x_all = x.reshape(T, DIM).astype(BF16)
    wo = W_out.astype(BF16)
    bo = b_out.astype(np.float32).reshape(1, DIM)
    in_maps = []
    for i in range(NCORES):
        c0 = i * CHC  # channel block of this core's 2 heads
        wq = Wf[:, 0 * DIM + c0 : 0 * DIM + c0 + CHC]
        wk = Wf[:, 1 * DIM + c0 : 1 * DIM + c0 + CHC]
        wv = Wf[:, 2 * DIM + c0 : 2 * DIM + c0 + CHC]
        bq = bf[0 * DIM + c0 : 0 * DIM + c0 + CHC]
        bk = bf[1 * DIM + c0 : 1 * DIM + c0 + CHC]
        bv = bf[2 * DIM + c0 : 2 * DIM + c0 + CHC]
        bqk = np.stack([bq, bk], axis=1).astype(np.float32)  # [128, 2]
        in_maps.append(
            {
                "x": x_all,
                "wq": np.ascontiguousarray(wq.astype(BF16)),
                "wk": np.ascontiguousarray(wk.astype(BF16)),
                "wv": np.ascontiguousarray(wv.astype(BF16)),
                "bqk": np.ascontiguousarray(bqk),
                "bv": np.ascontiguousarray(bv.astype(BF16).reshape(1, CHC)),
                "wo": wo,
                "bo": bo,
            }
        )
    return in_maps


def kernel(x, ln_gamma, ln_beta, W_qkv, W_out, b_out, _want_time=False):
    x = np.asarray(x, dtype=np.float32)
    ln_gamma = np.asarray(ln_gamma, dtype=np.float32)
    ln_beta = np.asarray(ln_beta, dtype=np.float32)
    W_qkv = np.asarray(W_qkv, dtype=np.float32)
    W_out = np.asarray(W_out, dtype=np.float32)
    b_out = np.asarray(b_out, dtype=np.float32)

    if "nc" not in _cache:
        _cache["nc"] = _build()
    nc = _cache["nc"]

    from concourse.bass_utils import run_bass_kernel_spmd

    in_maps = _prep_inputs(x, ln_gamma, ln_beta, W_qkv, W_out, b_out)
    res = run_bass_kernel_spmd(
        nc, in_maps, core_ids=list(range(NCORES)), trace=_want_time
    )
    out = np.empty((B, N, DIM), dtype=np.float32)
    for i in range(NCORES):
        b, g = i // 4, i % 4
        out[b, g * 512 : (g + 1) * 512, :] = res.results[i]["out"]
    if _want_time:
        return out, res.exec_time_ns
    return out
